# revision 10
# baseline (speedup 1.0000x reference)
"""Trainium2 Bass kernel for the vq_codebook clustering module.

Computation (per reference):
    e  = (z.reshape(B*SN, D) @ W1 + b1) @ W2 + b2          [B,SN,64]
    nrm[s,k] = || e[s,:] - centroids[k,:] ||_2
    s_un = 1 / (1 + nrm)            (Student-t, alpha=1)
    s  = s_un / sum_k s_un                                  [B,SN,100]
    c  = argmax_k s                                         [B,SN]

Key algebraic optimization: the two stacked Linears have no nonlinearity
between them, so  e = z @ Wc + bias2  with  Wc = W1 @ W2  ([12000, 64])
and bias2 = b1 @ W2 + b2.  This collapses the 50-GFLOP z@W1 matmul into
a 1.5-GFLOP Wc precompute plus a 3-GFLOP z@Wc matmul, moving the problem
from the ~170us/core roofline to ~60us/core.

Sharding (8 cores, single SPMD launch):
  phase A: W1 row-sharded — core i computes Wc[i*1500:(i+1)*1500, :]
           (transpose W1 tiles on the PE, 8 accumulating matmuls per tile)
  AllGather (gpsimd collective, 3 MB) -> every core holds full Wc
  phase B: samples row-sharded — core i computes e/s/c for its 256 samples:
           PE-transpose z tiles, accumulate e = z @ Wc in PSUM over 96
           chunks, then a small epilogue:
             eT via PE transpose; q = -2 e@centroidsT + ||c||^2 (ones-row
             matmul trick); nsq = q + ||e||^2; nrm = sqrt; r = 1/(1+nrm);
             s = r / rowsum(r); c = argmax via max/max_index.
All arithmetic is plain fp32 (matches the fp32 reference to ~1e-6).
"""

import numpy as np
from contextlib import ExitStack

import concourse.bass as bass
import concourse.bacc as bacc
import concourse.mybir as mybir
import concourse.tile as tile
from concourse.bass_utils import run_bass_kernel_spmd
from concourse.masks import make_identity

F32 = mybir.dt.float32
I32 = mybir.dt.int32
U32 = mybir.dt.uint32

N_CORES = 8
B, SN, D_IN = 32, 64, 12000
NDF1, NDF2 = 1024, 64
K_CL = 100
S_TOT = B * SN                      # 2048
S_PER = S_TOT // N_CORES            # 256
M_TILES = S_PER // 128              # 2
DSLICE = D_IN // N_CORES            # 1500 rows of W1 per core
A_T = (DSLICE + 127) // 128         # 12 d-tiles in phase A (last = 92 rows)
A_LAST = DSLICE - (A_T - 1) * 128   # 92
GC_TOT = N_CORES * A_T              # 96 global 128-chunks of padded Wc
FCH = NDF1 // 128                   # 8 f-chunks


def _build_program():
    nc = bacc.Bacc()

    z = nc.declare_dram_parameter("z", [S_PER, D_IN], F32, isOutput=False)
    w1s = nc.declare_dram_parameter("w1s", [DSLICE, NDF1], F32, isOutput=False)
    w2 = nc.declare_dram_parameter("w2", [NDF1, NDF2], F32, isOutput=False)
    bias2 = nc.declare_dram_parameter("bias2", [1, NDF2], F32, isOutput=False)
    cm = nc.declare_dram_parameter("cm", [NDF2, K_CL], F32, isOutput=False)
    cn = nc.declare_dram_parameter("cn", [1, K_CL], F32, isOutput=False)
    e_out = nc.declare_dram_parameter("e_out", [S_PER, NDF2], F32, isOutput=True)
    s_out = nc.declare_dram_parameter("s_out", [S_PER, K_CL], F32, isOutput=True)
    c_out = nc.declare_dram_parameter("c_out", [S_PER, 1], I32, isOutput=True)

    with ExitStack() as ctx:
        tc = ctx.enter_context(tile.TileContext(nc))

        consts = ctx.enter_context(tc.tile_pool(name="consts", bufs=1))
        a_pool = ctx.enter_context(tc.tile_pool(name="ap", bufs=2))
        b_pool = ctx.enter_context(tc.tile_pool(name="bp", bufs=2))
        zt_pool = ctx.enter_context(tc.tile_pool(name="ztp", bufs=4))
        ztps_pool = ctx.enter_context(tc.tile_pool(name="ztps", bufs=2, space="PSUM"))
        psacc_pool = ctx.enter_context(tc.tile_pool(name="psacc", bufs=1, space="PSUM"))
        psq_pool = ctx.enter_context(tc.tile_pool(name="psq", bufs=2, space="PSUM"))
        epi_pool = ctx.enter_context(tc.tile_pool(name="epi", bufs=1))
        dram = ctx.enter_context(tc.tile_pool(name="dram", bufs=1, space="DRAM"))

        # ---- constants ----
        identity = consts.tile([128, 128], F32, name="identity")
        make_identity(nc, identity[:])

        w2_sb = consts.tile([128, FCH, NDF2], F32, name="w2_sb")
        nc.sync.dma_start(w2_sb[:], w2.rearrange("(c p) g -> p c g", p=128))

        bias2_sb = consts.tile([1, NDF2], F32, name="bias2_sb")
        nc.sync.dma_start(bias2_sb[:], bias2[:])

        cm_sb = consts.tile([NDF2, K_CL], F32, name="cm_sb")
        nc.sync.dma_start(cm_sb[:], cm[:])

        cn_sb = consts.tile([1, K_CL], F32, name="cn_sb")
        nc.sync.dma_start(cn_sb[:], cn[:])

        ones_row = consts.tile([1, S_PER], F32, name="ones_row")
        nc.vector.memset(ones_row[:], 1.0)

        # ---- phase A: wc slice = w1s @ w2, padded to [A_T, 128, 64] ----
        wcs_dram = dram.tile([A_T, 128, NDF2], F32, name="wcs_dram")
        wc_all = dram.tile(
            [GC_TOT, 128, NDF2], F32, name="wc_all", addr_space="Shared"
        )

        for t in range(A_T):
            td = 128 if t < A_T - 1 else A_LAST
            w1s_t = a_pool.tile([128, NDF1], F32, name="w1s_t", tag="w1s")
            nc.sync.dma_start(w1s_t[:td, :], w1s[t * 128 : t * 128 + td, :])

            w1T = a_pool.tile([128, FCH, 128], F32, name="w1T", tag="w1T")
            for f in range(FCH):
                tp = ztps_pool.tile([128, 128], F32, name="tpA", tag="tp")
                nc.tensor.transpose(
                    tp[:, :td],
                    w1s_t[:td, f * 128 : (f + 1) * 128],
                    identity[:td, :td],
                )
                nc.vector.tensor_copy(w1T[:, f, :td], tp[:, :td])

            pwc = psq_pool.tile([128, NDF2], F32, name="pwc", tag="pwc")
            for f in range(FCH):
                nc.tensor.matmul(
                    pwc[:td, :],
                    lhsT=w1T[:, f, :td],
                    rhs=w2_sb[:, f, :],
                    start=(f == 0),
                    stop=(f == FCH - 1),
                )
            wcs_sb = a_pool.tile([128, NDF2], F32, name="wcs_sb", tag="wcs")
            if td < 128:
                nc.vector.memset(wcs_sb[:], 0.0)
            nc.vector.tensor_copy(wcs_sb[:td, :], pwc[:td, :])
            nc.sync.dma_start(wcs_dram[t, :, :], wcs_sb[:])

        # ---- AllGather wc slices ----
        nc.gpsimd.collective_compute(
            "AllGather",
            mybir.AluOpType.bypass,
            replica_groups=[list(range(N_CORES))],
            ins=[wcs_dram[:].opt()],
            outs=[wc_all[:].opt()],
        )

        wc_sb = consts.tile([128, GC_TOT, NDF2], F32, name="wc_sb")
        nc.sync.dma_start(wc_sb[:], wc_all[:].rearrange("c p g -> p c g"))

        # ---- phase B: e = z @ wc + bias2, accumulated over 96 chunks ----
        ps_e = [
            psacc_pool.tile([128, NDF2], F32, name=f"ps_e{m}", tag=f"ps_e{m}")
            for m in range(M_TILES)
        ]

        for ic in range(N_CORES):
            zlen = A_T * 128 if ic < N_CORES - 1 else DSLICE
            zb = []
            for m in range(M_TILES):
                t_ = b_pool.tile([128, A_T * 128], F32, name=f"zb{m}", tag=f"zb{m}")
                nc.sync.dma_start(
                    t_[:, :zlen],
                    z[m * 128 : (m + 1) * 128, ic * DSLICE : ic * DSLICE + zlen],
                )
                zb.append(t_)
            for t in range(A_T):
                gc = ic * A_T + t
                kd = 128 if gc < GC_TOT - 1 else A_LAST
                zt = zt_pool.tile([128, S_PER], F32, name="zt", tag="zt")
                for m in range(M_TILES):
                    tp = ztps_pool.tile([128, 128], F32, name="tpB", tag="tp")
                    nc.tensor.transpose(
                        tp[:kd, :], zb[m][:, t * 128 : t * 128 + kd], identity[:]
                    )
                    nc.vector.tensor_copy(
                        zt[:kd, m * 128 : (m + 1) * 128], tp[:kd, :]
                    )
                for m in range(M_TILES):
                    nc.tensor.matmul(
                        ps_e[m][:],
                        lhsT=zt[:kd, m * 128 : (m + 1) * 128],
                        rhs=wc_sb[:kd, gc, :],
                        start=(gc == 0),
                        stop=False,
                    )
        for m in range(M_TILES):
            nc.tensor.matmul(
                ps_e[m][:],
                lhsT=ones_row[:1, :128],
                rhs=bias2_sb[:1, :],
                start=False,
                stop=True,
            )

        # ---- epilogue ----
        e_sb = []
        for m in range(M_TILES):
            t_ = epi_pool.tile([128, NDF2], F32, name=f"e_sb{m}")
            nc.vector.tensor_copy(t_[:], ps_e[m][:])
            e_sb.append(t_)
            nc.sync.dma_start(e_out[m * 128 : (m + 1) * 128, :], t_[:])

        eT_sb = epi_pool.tile([NDF2, S_PER], F32, name="eT_sb")
        for m in range(M_TILES):
            tp = ztps_pool.tile([128, 128], F32, name="tpE", tag="tp")
            nc.tensor.transpose(tp[:NDF2, :], e_sb[m][:], identity[:])
            nc.vector.tensor_copy(eT_sb[:, m * 128 : (m + 1) * 128], tp[:NDF2, :])

        for m in range(M_TILES):
            pq = psq_pool.tile([128, K_CL], F32, name=f"pq{m}", tag="pwc")
            nc.tensor.matmul(
                pq[:],
                lhsT=eT_sb[:, m * 128 : (m + 1) * 128],
                rhs=cm_sb[:],
                start=True,
                stop=False,
            )
            nc.tensor.matmul(
                pq[:],
                lhsT=ones_row[:1, :128],
                rhs=cn_sb[:1, :],
                start=False,
                stop=True,
            )

            esq_scratch = epi_pool.tile([128, NDF2], F32, name=f"esqs{m}")
            esq = epi_pool.tile([128, 1], F32, name=f"esq{m}")
            nc.vector.tensor_mul(esq_scratch[:], e_sb[m][:], e_sb[m][:])
            nc.vector.tensor_reduce(
                esq[:], esq_scratch[:], mybir.AxisListType.X, mybir.AluOpType.add
            )

            nsq = epi_pool.tile([128, K_CL], F32, name=f"nsq{m}")
            nc.vector.tensor_scalar_add(nsq[:], pq[:], esq[:])

            nrm = epi_pool.tile([128, K_CL], F32, name=f"nrm{m}")
            nc.scalar.sqrt(nrm[:], nsq[:])

            nrm1 = epi_pool.tile([128, K_CL], F32, name=f"nrm1{m}")
            nc.vector.tensor_scalar_add(nrm1[:], nrm[:], 1.0)

            r = epi_pool.tile([128, K_CL], F32, name=f"r{m}")
            nc.vector.reciprocal(r[:], nrm1[:])

            rowsum = epi_pool.tile([128, 1], F32, name=f"rowsum{m}")
            nc.vector.tensor_reduce(
                rowsum[:], r[:], mybir.AxisListType.X, mybir.AluOpType.add
            )
            rinv = epi_pool.tile([128, 1], F32, name=f"rinv{m}")
            nc.vector.reciprocal(rinv[:], rowsum[:])

            s_sb = epi_pool.tile([128, K_CL], F32, name=f"s_sb{m}")
            nc.vector.tensor_scalar_mul(s_sb[:], r[:], rinv[:])
            nc.sync.dma_start(s_out[m * 128 : (m + 1) * 128, :], s_sb[:])

            max8 = epi_pool.tile([128, 8], F32, name=f"max8{m}")
            nc.vector.max(max8[:], r[:])
            idx8 = epi_pool.tile([128, 8], U32, name=f"idx8{m}")
            nc.vector.max_index(idx8[:], max8[:], r[:])
            nc.sync.dma_start(
                c_out[m * 128 : (m + 1) * 128, :], idx8[:, 0:1].bitcast(I32)
            )

    nc.finalize()
    return nc


_CACHE = {}


def _get_program():
    if "prog" not in _CACHE:
        _CACHE["prog"] = _build_program()
    return _CACHE["prog"]


def _make_in_maps(z, W1, b1, W2, b2, centroids):
    zf = np.ascontiguousarray(z.reshape(S_TOT, D_IN))
    bias2 = (
        b1.astype(np.float64) @ W2.astype(np.float64) + b2.astype(np.float64)
    ).astype(np.float32).reshape(1, NDF2)
    cm = np.ascontiguousarray((-2.0 * centroids.T).astype(np.float32))
    cn = (
        (centroids.astype(np.float64) ** 2).sum(axis=1).astype(np.float32)
    ).reshape(1, K_CL)

    in_maps = []
    for i in range(N_CORES):
        in_maps.append(
            {
                "z": zf[i * S_PER : (i + 1) * S_PER],
                "w1s": W1[i * DSLICE : (i + 1) * DSLICE],
                "w2": W2,
                "bias2": bias2,
                "cm": cm,
                "cn": cn,
            }
        )
    return in_maps


def _assemble(results):
    e = np.concatenate([r["e_out"] for r in results], axis=0)
    s = np.concatenate([r["s_out"] for r in results], axis=0)
    c = np.concatenate([r["c_out"] for r in results], axis=0)
    e = e.reshape(B, SN, NDF2)
    s = s.reshape(B, SN, K_CL)
    c = c.reshape(B, SN).astype(np.int32)
    return e, s, c


def kernel(z, W1, b1, W2, b2, centroids, _trace=False):
    z = np.asarray(z, dtype=np.float32)
    W1 = np.asarray(W1, dtype=np.float32)
    b1 = np.asarray(b1, dtype=np.float32)
    W2 = np.asarray(W2, dtype=np.float32)
    b2 = np.asarray(b2, dtype=np.float32)
    centroids = np.asarray(centroids, dtype=np.float32)

    nc = _get_program()
    in_maps = _make_in_maps(z, W1, b1, W2, b2, centroids)

    res = run_bass_kernel_spmd(
        nc, in_maps, core_ids=list(range(N_CORES)), trace=_trace
    )
    _CACHE["last_exec_time_ns"] = res.exec_time_ns
    _CACHE["last_results"] = res

    return _assemble(res.results)
